# revision 43
# baseline (speedup 1.0000x reference)
"""Multi-head causal attention (B=4, S=2048, D=1024, H=16) on 8 TRN2 cores.

Sharding: tensor-parallel over heads (2 heads/core), proj_out row-parallel
with the cross-core reduction done host-side during unsharding.

Per-core kernel layout (all contractions on the SBUF partition axis):
  xT      (1024 d, 8192 tok)   host-pretransposed activations (shared input)
  qT/kT   (128 e2, 2048 s)     per batch; e2 = 2 heads x 64
  scoresT (128 sk, 512 sq)     kv-major scores -> exp -> PV matmul directly
  denom   ones-matmul broadcast of the per-column sums of exp(scores)
  ctxT    (128 e2, 512 sq)     normalized, fed straight into row-parallel Wo
  outp    (1024 o, 8192 tok)   per-core partial; host sums over cores

Head pairs run concurrently on the PE via automatic tile_position (row
tiles for the 64-contraction score matmuls, col tiles for the 64-wide
den/PV matmuls).  Pipeline: warmup matmuls keep the HAM clock-gate warm
through the initial DMA; qkv projection + deferred output-projection
work is interleaved at fine grain between attention iterations so the
PE never idles long enough to re-throttle.
"""

import sys

if "/opt/trn_rl_repo" not in sys.path:
    sys.path.insert(0, "/opt/trn_rl_repo")

from contextlib import ExitStack

import numpy as np

import concourse.bass as bass
import concourse.bacc as bacc
import concourse.mybir as mybir
import concourse.tile as tile
from concourse.bass_utils import run_bass_kernel_spmd
from concourse.masks import make_identity

B, S, D, H, E = 4, 2048, 1024, 16, 64
NCORES = 8
HL = H // NCORES          # heads per core = 2
EL = HL * E               # local feature width = 128
SQ = 512                  # query chunk (matmul moving dim)
NQ = S // SQ              # 4
KT = 128                  # kv tile (contraction tile)
DT = 128                  # d-model contraction tile
ND = D // DT              # 8
N_WARM = 16               # HAM warmup matmuls during initial DMA
F32 = mybir.dt.float32
BF16 = mybir.dt.bfloat16
EXP = mybir.ActivationFunctionType.Exp

LAST_RESULTS = None


def build():
    nc = bacc.Bacc()
    xT = nc.declare_dram_parameter("xT", [B * NQ, DT, ND, SQ], BF16, isOutput=False)
    wqkv = nc.declare_dram_parameter("wqkv", [DT, ND, 3 * EL], BF16, isOutput=False)
    wo = nc.declare_dram_parameter("wo", [EL, D], BF16, isOutput=False)
    masks = nc.declare_dram_parameter("masks", [KT, NQ, 2, SQ], BF16, isOutput=False)
    outp = nc.declare_dram_parameter("outp", [D, B * S], BF16, isOutput=True)

    with tile.TileContext(nc) as tc, ExitStack() as ctx:
        consts = ctx.enter_context(tc.tile_pool(name="consts", bufs=1))
        xt_pool = ctx.enter_context(tc.tile_pool(name="xt", bufs=3))
        qk_pool = ctx.enter_context(tc.tile_pool(name="qk", bufs=2))
        ex_pool = ctx.enter_context(tc.tile_pool(name="ex", bufs=6))
        misc_pool = ctx.enter_context(tc.tile_pool(name="misc", bufs=2))
        out_pool = ctx.enter_context(tc.tile_pool(name="outsb", bufs=8))
        mm_psum = ctx.enter_context(tc.tile_pool(name="mmps", bufs=2, space="PSUM"))
        sc_psum = ctx.enter_context(tc.tile_pool(name="scps", bufs=2, space="PSUM"))
        acc_psum = ctx.enter_context(tc.tile_pool(name="accps", bufs=1, space="PSUM"))
        den_psum = ctx.enter_context(tc.tile_pool(name="denps", bufs=1, space="PSUM"))

        # DMA queue plan for the startup window: the sync (HWDGE) queue
        # carries wqkv (host-pretransposed so the transfer is contiguous)
        # then the first x chunk; masks/wo are only needed a few us in so
        # they load after the first chunks.
        wqkv_sb = consts.tile([DT, ND, 3 * EL], BF16)
        nc.sync.dma_start(wqkv_sb[:], wqkv[:])
        masks_sb = consts.tile([KT, NQ, 2, SQ], BF16)
        wo_sb = consts.tile([EL, D], BF16)
        ones_sb = consts.tile([DT, DT], BF16)
        nc.vector.memset(ones_sb[:], 1.0)
        ident = consts.tile([DT, DT], BF16)
        make_identity(nc, ident[:])

        # HAM warmup: keep the PE busy through the initial DMA window so
        # the clock-gate is at 8/8 when the first real matmul issues.
        warm_src = consts.tile([DT, SQ], BF16)
        nc.vector.memset(warm_src[:], 0.0)
        warm_ps = mm_psum.tile([DT, SQ], F32, name="warm_ps", tag="mm")
        for _ in range(N_WARM):
            nc.tensor.matmul(warm_ps[:], ones_sb[:], warm_src[:], start=True, stop=True)

        qkv_tiles = {}

        def qkv_chunk_pieces(b, c):
            # returns fine-grained filler closures; each emits a small piece
            # of the qkv work for chunk (b, c) so it can be sprinkled between
            # attention iterations (engine FIFOs are strict in-order)
            if c == 0:
                qT = qk_pool.tile([EL, S], BF16, name=f"qT_{b}", tag="qT")
                kT = qk_pool.tile([EL, S], BF16, name=f"kT_{b}", tag="kT")
                vT = qk_pool.tile([EL, S], BF16, name=f"vT_{b}", tag="vT", bufs=1)
                v_sb = qk_pool.tile([KT, S // KT, EL], BF16, name=f"v_{b}", tag="v")
                qkv_tiles[b] = (qT, kT, vT, v_sb)
            qT, kT, vT, v_sb = qkv_tiles[b]
            xt8 = xt_pool.tile([DT, ND, SQ], BF16, name=f"xt_{b}_{c}", tag="xt")
            # x is host-prechunked to [chunk, p, t, n] so each load is one
            # contiguous 8KB-per-partition transfer; split across HWDGE
            # (sync) and SWDGE (gpsimd) so the two halves stream in
            # parallel and the t<4 matmuls start after half has landed
            half = ND // 2
            u = b * NQ + c
            nc.sync.dma_start(xt8[:, 0:half, :], xT[u, :, 0:half, :])
            nc.gpsimd.dma_start(xt8[:, half:ND, :], xT[u, :, half:ND, :])

            psums = {}

            def proj_piece(dest, col0, t0, t1):
                def go():
                    if t0 == 0:
                        psums[col0] = mm_psum.tile(
                            [EL, SQ], F32, name=f"qkv_ps_{b}_{c}_{col0}", tag="mm"
                        )
                    ps = psums[col0]
                    for t in range(t0, t1):
                        nc.tensor.matmul(
                            ps[:],
                            wqkv_sb[:, t, col0:col0 + EL],
                            xt8[:, t, :],
                            start=(t == 0),
                            stop=(t == ND - 1),
                        )
                    if t1 == ND:
                        nc.vector.tensor_copy(dest[:, c * SQ:(c + 1) * SQ], ps[:])
                return go

            def vtr(j0):
                def go():
                    for j in (j0, j0 + 1):
                        vt_ps = mm_psum.tile([KT, KT], BF16, name=f"vt_ps_{b}_{j}", tag="mm")
                        nc.tensor.transpose(vt_ps[:], vT[:, j * KT:(j + 1) * KT], ident[:])
                        nc.vector.tensor_copy(v_sb[:, j, :], vt_ps[:])
                return go

            pieces = []
            for col0, dest in ((0, qT), (EL, kT), (2 * EL, vT)):
                for t0 in range(0, ND, 4):
                    pieces.append(proj_piece(dest, col0, t0, t0 + 4))
            pieces.append(vtr(4 * c))
            pieces.append(vtr(4 * c + 2))
            return pieces

        # global filler queue: (chunk_tag_or_None, closure).  Chunk pieces
        # and deferred output projections pop between attention iterations
        # at a self-balancing cadence; `reserve` pieces are held back so
        # later units never starve.
        fill_q = []

        def pop_one():
            if fill_q:
                fill_q.pop(0)[1]()

        def emit_attn_unit(b, c, reserve=0, last=False):
            # returns tail closures (the row-parallel output projection) to
            # be deferred into later units' iteration loops
            qT, kT, vT, v_sb = qkv_tiles[b]
            J = (c + 1) * (SQ // KT)  # causal kv tiles for this chunk
            ctx_ps = acc_psum.tile([2 * E, SQ], F32, name=f"ctx_{b}_{c}", tag="ctx")
            denb = den_psum.tile([KT, SQ], F32, name=f"den_{b}_{c}", tag="den")
            def emit_denpv(j, ex, cut):
                # denominator rides PE: ones.T @ ex accumulates the
                # per-column sums, already broadcast over partitions.
                for h in range(HL):
                    nc.tensor.matmul(
                        denb[h * E:(h + 1) * E, cut:SQ],
                        ones_sb[:, h * E:(h + 1) * E],
                        ex[:, h, cut:SQ],
                        start=(j == 0),
                        stop=(j == J - 1),
                        skip_group_check=True,
                    )
                for h in range(HL):
                    nc.tensor.matmul(
                        ctx_ps[h * E:(h + 1) * E, cut:SQ],
                        v_sb[:, j, h * E:(h + 1) * E],
                        ex[:, h, cut:SQ],
                        start=(j == 0),
                        stop=(j == J - 1),
                        skip_group_check=True,
                    )

            pending = None  # den/PV lag the scores by one iteration
            for j in range(J):
                rdiag = j - (c * (SQ // KT))
                # columns [0, cut) of this q-chunk are fully masked for
                # diagonal kv tiles -- skip them everywhere
                cut = KT * rdiag if rdiag > 0 else 0
                n = SQ - cut
                sc = sc_psum.tile([KT, 2, SQ], F32, name=f"sc_{b}_{c}_{j}", tag="sc")
                ex = ex_pool.tile([KT, 2, SQ], BF16, name=f"ex_{b}_{c}_{j}", tag="ex")
                for h in range(HL):
                    nc.tensor.matmul(
                        sc[:, h, 0:n],
                        kT[h * E:(h + 1) * E, j * KT:(j + 1) * KT],
                        qT[h * E:(h + 1) * E, c * SQ + cut:(c + 1) * SQ],
                        start=True,
                        stop=True,
                    )
                nc.scalar.activation(
                    ex[:, :, cut:SQ], sc[:, :, 0:n], EXP, scale=0.125
                )
                if rdiag >= 0:
                    nc.vector.tensor_mul(
                        ex[:, :, cut:SQ],
                        ex[:, :, cut:SQ],
                        masks_sb[:, rdiag, :, cut:SQ],
                    )
                # one filler here keeps the PE streaming while the exp
                # for this iteration drains through the scalar engine
                pop_one()
                if pending is not None:
                    emit_denpv(*pending)
                pending = (j, ex, cut)
                # self-balancing filler cadence: spread the queue (minus
                # the held-back reserve) evenly over remaining iterations;
                # the final unit drains early so its normalize chain isn't
                # queued behind leftover copies
                horizon = max(1, J - j - 2) if last else (J - j)
                quota = -(-max(0, len(fill_q) - reserve) // horizon) - 1
                for _ in range(quota):
                    pop_one()
            emit_denpv(*pending)

            recb = misc_pool.tile([KT, SQ], F32, name=f"rec_{b}_{c}", tag="recb")
            nc.vector.reciprocal_approx_fast(recb[:], denb[:])
            ctx_sb = misc_pool.tile(
                [2 * E, SQ], BF16, name=f"ctxsb_{b}_{c}", tag="ctxsb", bufs=3
            )
            nc.vector.tensor_mul(ctx_sb[:], ctx_ps[:], recb[:])

            # ---- row-parallel output projection (partial), deferred ----
            # These pieces pop during later units.  When they land in an
            # ACT-idle phase (after a c==3 unit, or the final flush), the
            # PSUM->SBUF copy goes to the scalar engine so the vector
            # engine's copy backlog doesn't stall the mm_psum rotation;
            # the final flush also borrows the (now idle) score banks.
            def oproj_piece(o, scalar_copy=False, use_sc_psum=False):
                def go():
                    pool = sc_psum if use_sc_psum else mm_psum
                    tag = "sc" if use_sc_psum else "mm"
                    ops = pool.tile([DT, SQ], F32, name=f"op_{b}_{c}_{o}", tag=tag)
                    nc.tensor.matmul(
                        ops[:], wo_sb[:, o * DT:(o + 1) * DT], ctx_sb[:],
                        start=True, stop=True,
                    )
                    osb = out_pool.tile([DT, SQ], BF16, name=f"osb_{b}_{c}_{o}", tag="osb")
                    if scalar_copy:
                        nc.scalar.activation(
                            osb[:], ops[:], mybir.ActivationFunctionType.Copy
                        )
                    else:
                        nc.vector.tensor_copy(osb[:], ops[:])
                    nc.sync.dma_start(
                        outp[o * DT:(o + 1) * DT, b * S + c * SQ: b * S + (c + 1) * SQ],
                        osb[:],
                    )
                return go

            return [
                oproj_piece(
                    o,
                    scalar_copy=(last and o % 2 == 1),
                    use_sc_psum=(last and o % 2 == 1),
                )
                for o in range(D // DT)
            ]

        # software pipeline: the global queue runs two qkv chunks ahead of
        # the attention units, plus deferred output projections.  The last
        # batch's units are rotated so the final unit is a small one (J=4)
        # and the kernel tail stays dense.
        NU = B * NQ
        unit_order = [(b, c) for b in range(B) for c in range(NQ)]
        unit_order = unit_order[:-NQ] + unit_order[-NQ + 1:] + [unit_order[-NQ]]
        chunk_order = [(b, c) for b in range(B) for c in range(NQ)]

        for p in qkv_chunk_pieces(0, 0):
            p()
        nc.gpsimd.dma_start(masks_sb[:], masks[:])
        nc.sync.dma_start(wo_sb[:], wo[:])
        fill_q += [((0, 1), p) for p in qkv_chunk_pieces(0, 1)]

        for i, (b, c) in enumerate(unit_order):
            if i + 2 < NU:
                ch = chunk_order[i + 2]
                fill_q += [(ch, p) for p in qkv_chunk_pieces(*ch)]
            # guard: every chunk this unit reads must be emitted before
            # the unit's first score matmul
            while any(
                t is not None and t[0] == b and t[1] <= c for t, _ in fill_q
            ):
                pop_one()
            tail = emit_attn_unit(
                b, c, reserve=8 if i < NU - 2 else 0, last=(i == NU - 1)
            )
            fill_q += [(None, p) for p in tail]
        while fill_q:
            pop_one()

    nc.finalize()
    return nc


def _host_inputs(x, Wq, Wk, Wv, Wo):
    import ml_dtypes

    bf = ml_dtypes.bfloat16
    # [chunk, p, t, n]: per-chunk contiguous tiles of x^T
    xT = np.ascontiguousarray(
        x.reshape(B * NQ, SQ, ND, DT).transpose(0, 3, 2, 1)
    ).astype(bf)
    p = np.arange(KT)[:, None, None]
    rr = np.arange(NQ)[None, :, None]
    cc = np.arange(SQ)[None, None, :]
    masks = (cc >= KT * rr + p).astype(bf)
    # duplicated per head so the mask multiply is one [KT, 2, n] DVE op
    masks = np.ascontiguousarray(np.repeat(masks[:, :, None, :], 2, axis=2))
    in_maps = []
    for core in range(NCORES):
        hs = slice(core * HL, (core + 1) * HL)
        wq = Wq[hs].reshape(EL, D).T
        wk = Wk[hs].reshape(EL, D).T
        wv = Wv[hs].reshape(EL, D).T
        wqkv = np.ascontiguousarray(
            np.concatenate([wq, wk, wv], axis=1)
            .reshape(ND, DT, 3 * EL)
            .transpose(1, 0, 2)
        ).astype(bf)
        woL = np.ascontiguousarray(
            Wo[:, core * EL:(core + 1) * EL].T
        ).astype(bf)
        in_maps.append({"xT": xT, "wqkv": wqkv, "wo": woL, "masks": masks})
    return in_maps


def kernel(x, Wq, Wk, Wv, Wo):
    global LAST_RESULTS
    x, Wq, Wk, Wv, Wo = (np.asarray(a, dtype=np.float32) for a in (x, Wq, Wk, Wv, Wo))
    nc = build()
    in_maps = _host_inputs(x, Wq, Wk, Wv, Wo)
    import os
    res = run_bass_kernel_spmd(
        nc, in_maps, list(range(NCORES)),
        trace=bool(os.environ.get("BASS_KERNEL_TRACE")),
    )
    LAST_RESULTS = res
    acc = np.zeros((D, B * S), np.float32)
    for rmap in res.results:
        acc += rmap["outp"]
    return np.ascontiguousarray(acc.T).reshape(B, S, D)


if __name__ == "__main__":
    rng = np.random.default_rng(0)
    scale = 1.0 / np.sqrt(D)
    x = rng.standard_normal((B, S, D), dtype=np.float32)
    Wq = rng.standard_normal((H, E, D), dtype=np.float32) * scale
    Wk = rng.standard_normal((H, E, D), dtype=np.float32) * scale
    Wv = rng.standard_normal((H, E, D), dtype=np.float32) * scale
    Wo = rng.standard_normal((D, D), dtype=np.float32) * scale
    out = kernel(x, Wq, Wk, Wv, Wo)
    print(out.shape, out.dtype, float(np.abs(out).max()))


# revision 44
# speedup vs baseline: 1.0011x; 1.0011x over previous
"""Multi-head causal attention (B=4, S=2048, D=1024, H=16) on 8 TRN2 cores.

Sharding: tensor-parallel over heads (2 heads/core), proj_out row-parallel
with the cross-core reduction done host-side during unsharding.

Per-core kernel layout (all contractions on the SBUF partition axis):
  xT      (1024 d, 8192 tok)   host-pretransposed activations (shared input)
  qT/kT   (128 e2, 2048 s)     per batch; e2 = 2 heads x 64
  scoresT (128 sk, 512 sq)     kv-major scores -> exp -> PV matmul directly
  denom   ones-matmul broadcast of the per-column sums of exp(scores)
  ctxT    (128 e2, 512 sq)     normalized, fed straight into row-parallel Wo
  outp    (1024 o, 8192 tok)   per-core partial; host sums over cores

Head pairs run concurrently on the PE via automatic tile_position (row
tiles for the 64-contraction score matmuls, col tiles for the 64-wide
den/PV matmuls).  Pipeline: warmup matmuls keep the HAM clock-gate warm
through the initial DMA; qkv projection + deferred output-projection
work is interleaved at fine grain between attention iterations so the
PE never idles long enough to re-throttle.
"""

import sys

if "/opt/trn_rl_repo" not in sys.path:
    sys.path.insert(0, "/opt/trn_rl_repo")

from contextlib import ExitStack

import numpy as np

import concourse.bass as bass
import concourse.bacc as bacc
import concourse.mybir as mybir
import concourse.tile as tile
from concourse.bass_utils import run_bass_kernel_spmd
from concourse.masks import make_identity

B, S, D, H, E = 4, 2048, 1024, 16, 64
NCORES = 8
HL = H // NCORES          # heads per core = 2
EL = HL * E               # local feature width = 128
SQ = 512                  # query chunk (matmul moving dim)
NQ = S // SQ              # 4
KT = 128                  # kv tile (contraction tile)
DT = 128                  # d-model contraction tile
ND = D // DT              # 8
N_WARM = 16               # HAM warmup matmuls during initial DMA
F32 = mybir.dt.float32
BF16 = mybir.dt.bfloat16
EXP = mybir.ActivationFunctionType.Exp

LAST_RESULTS = None


def build():
    nc = bacc.Bacc()
    xT = nc.declare_dram_parameter("xT", [B * NQ, DT, ND, SQ], BF16, isOutput=False)
    wqkv = nc.declare_dram_parameter("wqkv", [DT, ND, 3 * EL], BF16, isOutput=False)
    wo = nc.declare_dram_parameter("wo", [EL, D], BF16, isOutput=False)
    masks = nc.declare_dram_parameter("masks", [KT, NQ, 2, SQ], BF16, isOutput=False)
    outp = nc.declare_dram_parameter("outp", [D, B * S], BF16, isOutput=True)

    with tile.TileContext(nc) as tc, ExitStack() as ctx:
        consts = ctx.enter_context(tc.tile_pool(name="consts", bufs=1))
        xt_pool = ctx.enter_context(tc.tile_pool(name="xt", bufs=3))
        qk_pool = ctx.enter_context(tc.tile_pool(name="qk", bufs=2))
        ex_pool = ctx.enter_context(tc.tile_pool(name="ex", bufs=6))
        misc_pool = ctx.enter_context(tc.tile_pool(name="misc", bufs=2))
        out_pool = ctx.enter_context(tc.tile_pool(name="outsb", bufs=8))
        mm_psum = ctx.enter_context(tc.tile_pool(name="mmps", bufs=2, space="PSUM"))
        sc_psum = ctx.enter_context(tc.tile_pool(name="scps", bufs=2, space="PSUM"))
        acc_psum = ctx.enter_context(tc.tile_pool(name="accps", bufs=1, space="PSUM"))
        den_psum = ctx.enter_context(tc.tile_pool(name="denps", bufs=1, space="PSUM"))

        # DMA queue plan for the startup window: the sync (HWDGE) queue
        # carries wqkv (host-pretransposed so the transfer is contiguous)
        # then the first x chunk; masks/wo are only needed a few us in so
        # they load after the first chunks.
        wqkv_sb = consts.tile([DT, ND, 3 * EL], BF16)
        nc.sync.dma_start(wqkv_sb[:], wqkv[:])
        masks_sb = consts.tile([KT, NQ, 2, SQ], BF16)
        wo_sb = consts.tile([EL, D], BF16)
        ones_sb = consts.tile([DT, DT], BF16)
        nc.vector.memset(ones_sb[:], 1.0)
        ident = consts.tile([DT, DT], BF16)
        make_identity(nc, ident[:])

        # HAM warmup: keep the PE busy through the initial DMA window so
        # the clock-gate is at 8/8 when the first real matmul issues.
        warm_src = consts.tile([DT, SQ], BF16)
        nc.vector.memset(warm_src[:], 0.0)
        warm_ps = mm_psum.tile([DT, SQ], F32, name="warm_ps", tag="mm")
        for _ in range(N_WARM):
            nc.tensor.matmul(warm_ps[:], ones_sb[:], warm_src[:], start=True, stop=True)

        qkv_tiles = {}

        def qkv_chunk_pieces(b, c):
            # returns fine-grained filler closures; each emits a small piece
            # of the qkv work for chunk (b, c) so it can be sprinkled between
            # attention iterations (engine FIFOs are strict in-order)
            if c == 0:
                qT = qk_pool.tile([EL, S], BF16, name=f"qT_{b}", tag="qT")
                kT = qk_pool.tile([EL, S], BF16, name=f"kT_{b}", tag="kT")
                vT = qk_pool.tile([EL, S], BF16, name=f"vT_{b}", tag="vT", bufs=1)
                v_sb = qk_pool.tile([KT, S // KT, EL], BF16, name=f"v_{b}", tag="v")
                qkv_tiles[b] = (qT, kT, vT, v_sb)
            qT, kT, vT, v_sb = qkv_tiles[b]
            xt8 = xt_pool.tile([DT, ND, SQ], BF16, name=f"xt_{b}_{c}", tag="xt")
            # x is host-prechunked to [chunk, p, t, n] so each load is one
            # contiguous 8KB-per-partition transfer; split across HWDGE
            # (sync) and SWDGE (gpsimd) so the two halves stream in
            # parallel and the t<4 matmuls start after half has landed
            half = ND // 2
            u = b * NQ + c
            nc.sync.dma_start(xt8[:, 0:half, :], xT[u, :, 0:half, :])
            nc.gpsimd.dma_start(xt8[:, half:ND, :], xT[u, :, half:ND, :])

            psums = {}

            def proj_piece(dest, col0, t0, t1):
                def go():
                    if t0 == 0:
                        psums[col0] = mm_psum.tile(
                            [EL, SQ], F32, name=f"qkv_ps_{b}_{c}_{col0}", tag="mm"
                        )
                    ps = psums[col0]
                    for t in range(t0, t1):
                        nc.tensor.matmul(
                            ps[:],
                            wqkv_sb[:, t, col0:col0 + EL],
                            xt8[:, t, :],
                            start=(t == 0),
                            stop=(t == ND - 1),
                        )
                    if t1 == ND:
                        nc.vector.tensor_copy(dest[:, c * SQ:(c + 1) * SQ], ps[:])
                return go

            def vtr(j0):
                def go():
                    for j in (j0, j0 + 1):
                        vt_ps = mm_psum.tile([KT, KT], BF16, name=f"vt_ps_{b}_{j}", tag="mm")
                        nc.tensor.transpose(vt_ps[:], vT[:, j * KT:(j + 1) * KT], ident[:])
                        nc.vector.tensor_copy(v_sb[:, j, :], vt_ps[:])
                return go

            pieces = []
            for col0, dest in ((0, qT), (EL, kT), (2 * EL, vT)):
                for t0 in range(0, ND, 4):
                    pieces.append(proj_piece(dest, col0, t0, t0 + 4))
            pieces.append(vtr(4 * c))
            pieces.append(vtr(4 * c + 2))
            return pieces

        # global filler queue: (chunk_tag_or_None, closure).  Chunk pieces
        # and deferred output projections pop between attention iterations
        # at a self-balancing cadence; `reserve` pieces are held back so
        # later units never starve.
        fill_q = []

        def pop_one():
            if fill_q:
                fill_q.pop(0)[1]()

        def emit_attn_unit(b, c, reserve=0, last=False):
            # returns tail closures (the row-parallel output projection) to
            # be deferred into later units' iteration loops
            qT, kT, vT, v_sb = qkv_tiles[b]
            J = (c + 1) * (SQ // KT)  # causal kv tiles for this chunk
            ctx_ps = acc_psum.tile([2 * E, SQ], F32, name=f"ctx_{b}_{c}", tag="ctx")
            denb = den_psum.tile([KT, SQ], F32, name=f"den_{b}_{c}", tag="den")
            def emit_denpv(j, ex, cut):
                # denominator rides PE: ones.T @ ex accumulates the
                # per-column sums, already broadcast over partitions.
                for h in range(HL):
                    nc.tensor.matmul(
                        denb[h * E:(h + 1) * E, cut:SQ],
                        ones_sb[:, h * E:(h + 1) * E],
                        ex[:, h, cut:SQ],
                        start=(j == 0),
                        stop=(j == J - 1),
                        skip_group_check=True,
                    )
                for h in range(HL):
                    nc.tensor.matmul(
                        ctx_ps[h * E:(h + 1) * E, cut:SQ],
                        v_sb[:, j, h * E:(h + 1) * E],
                        ex[:, h, cut:SQ],
                        start=(j == 0),
                        stop=(j == J - 1),
                        skip_group_check=True,
                    )

            pending = None  # den/PV lag the scores by one iteration
            for j in range(J):
                rdiag = j - (c * (SQ // KT))
                # columns [0, cut) of this q-chunk are fully masked for
                # diagonal kv tiles -- skip them everywhere
                cut = KT * rdiag if rdiag > 0 else 0
                n = SQ - cut
                sc = sc_psum.tile([KT, 2, SQ], F32, name=f"sc_{b}_{c}_{j}", tag="sc")
                ex = ex_pool.tile([KT, 2, SQ], BF16, name=f"ex_{b}_{c}_{j}", tag="ex")
                for h in range(HL):
                    nc.tensor.matmul(
                        sc[:, h, 0:n],
                        kT[h * E:(h + 1) * E, j * KT:(j + 1) * KT],
                        qT[h * E:(h + 1) * E, c * SQ + cut:(c + 1) * SQ],
                        start=True,
                        stop=True,
                    )
                nc.scalar.activation(
                    ex[:, :, cut:SQ], sc[:, :, 0:n], EXP, scale=0.125
                )
                if rdiag >= 0:
                    nc.vector.tensor_mul(
                        ex[:, :, cut:SQ],
                        ex[:, :, cut:SQ],
                        masks_sb[:, rdiag, :, cut:SQ],
                    )
                # one filler here keeps the PE streaming while the exp
                # for this iteration drains through the scalar engine
                pop_one()
                if pending is not None:
                    emit_denpv(*pending)
                pending = (j, ex, cut)
                # self-balancing filler cadence: spread the queue (minus
                # the held-back reserve) evenly over remaining iterations
                quota = -(-max(0, len(fill_q) - reserve) // (J - j)) - 1
                for _ in range(quota):
                    pop_one()
            emit_denpv(*pending)

            recb = misc_pool.tile([KT, SQ], F32, name=f"rec_{b}_{c}", tag="recb")
            nc.vector.reciprocal_approx_fast(recb[:], denb[:])
            ctx_sb = misc_pool.tile(
                [2 * E, SQ], BF16, name=f"ctxsb_{b}_{c}", tag="ctxsb", bufs=3
            )
            nc.vector.tensor_mul(ctx_sb[:], ctx_ps[:], recb[:])

            # ---- row-parallel output projection (partial), deferred ----
            # These pieces pop during later units.  When they land in an
            # ACT-idle phase (after a c==3 unit, or the final flush), the
            # PSUM->SBUF copy goes to the scalar engine so the vector
            # engine's copy backlog doesn't stall the mm_psum rotation;
            # the final flush also borrows the (now idle) score banks.
            def oproj_piece(o, scalar_copy=False, use_sc_psum=False):
                def go():
                    pool = sc_psum if use_sc_psum else mm_psum
                    tag = "sc" if use_sc_psum else "mm"
                    ops = pool.tile([DT, SQ], F32, name=f"op_{b}_{c}_{o}", tag=tag)
                    nc.tensor.matmul(
                        ops[:], wo_sb[:, o * DT:(o + 1) * DT], ctx_sb[:],
                        start=True, stop=True,
                    )
                    osb = out_pool.tile([DT, SQ], BF16, name=f"osb_{b}_{c}_{o}", tag="osb")
                    if scalar_copy:
                        nc.scalar.activation(
                            osb[:], ops[:], mybir.ActivationFunctionType.Copy
                        )
                    else:
                        nc.vector.tensor_copy(osb[:], ops[:])
                    nc.sync.dma_start(
                        outp[o * DT:(o + 1) * DT, b * S + c * SQ: b * S + (c + 1) * SQ],
                        osb[:],
                    )
                return go

            return [
                oproj_piece(
                    o,
                    scalar_copy=(last and o % 2 == 1),
                    use_sc_psum=(last and o % 2 == 1),
                )
                for o in range(D // DT)
            ]

        # software pipeline: the global queue runs two qkv chunks ahead of
        # the attention units, plus deferred output projections.  The last
        # batch's units are rotated so the final unit is a small one (J=4)
        # and the kernel tail stays dense.
        NU = B * NQ
        unit_order = [(b, c) for b in range(B) for c in range(NQ)]
        unit_order = unit_order[:-NQ] + unit_order[-NQ + 1:] + [unit_order[-NQ]]
        chunk_order = [(b, c) for b in range(B) for c in range(NQ)]

        for p in qkv_chunk_pieces(0, 0):
            p()
        nc.gpsimd.dma_start(masks_sb[:], masks[:])
        nc.sync.dma_start(wo_sb[:], wo[:])
        fill_q += [((0, 1), p) for p in qkv_chunk_pieces(0, 1)]

        for i, (b, c) in enumerate(unit_order):
            if i + 2 < NU:
                ch = chunk_order[i + 2]
                fill_q += [(ch, p) for p in qkv_chunk_pieces(*ch)]
            # guard: every chunk this unit reads must be emitted before
            # the unit's first score matmul
            while any(
                t is not None and t[0] == b and t[1] <= c for t, _ in fill_q
            ):
                pop_one()
            tail = emit_attn_unit(
                b, c, reserve=8 if i < NU - 2 else 0, last=(i == NU - 1)
            )
            fill_q += [(None, p) for p in tail]
        while fill_q:
            pop_one()

    nc.finalize()
    return nc


def _host_inputs(x, Wq, Wk, Wv, Wo):
    import ml_dtypes

    bf = ml_dtypes.bfloat16
    # [chunk, p, t, n]: per-chunk contiguous tiles of x^T
    xT = np.ascontiguousarray(
        x.reshape(B * NQ, SQ, ND, DT).transpose(0, 3, 2, 1)
    ).astype(bf)
    p = np.arange(KT)[:, None, None]
    rr = np.arange(NQ)[None, :, None]
    cc = np.arange(SQ)[None, None, :]
    masks = (cc >= KT * rr + p).astype(bf)
    # duplicated per head so the mask multiply is one [KT, 2, n] DVE op
    masks = np.ascontiguousarray(np.repeat(masks[:, :, None, :], 2, axis=2))
    in_maps = []
    for core in range(NCORES):
        hs = slice(core * HL, (core + 1) * HL)
        wq = Wq[hs].reshape(EL, D).T
        wk = Wk[hs].reshape(EL, D).T
        wv = Wv[hs].reshape(EL, D).T
        wqkv = np.ascontiguousarray(
            np.concatenate([wq, wk, wv], axis=1)
            .reshape(ND, DT, 3 * EL)
            .transpose(1, 0, 2)
        ).astype(bf)
        woL = np.ascontiguousarray(
            Wo[:, core * EL:(core + 1) * EL].T
        ).astype(bf)
        in_maps.append({"xT": xT, "wqkv": wqkv, "wo": woL, "masks": masks})
    return in_maps


def kernel(x, Wq, Wk, Wv, Wo):
    global LAST_RESULTS
    x, Wq, Wk, Wv, Wo = (np.asarray(a, dtype=np.float32) for a in (x, Wq, Wk, Wv, Wo))
    nc = build()
    in_maps = _host_inputs(x, Wq, Wk, Wv, Wo)
    import os
    res = run_bass_kernel_spmd(
        nc, in_maps, list(range(NCORES)),
        trace=bool(os.environ.get("BASS_KERNEL_TRACE")),
    )
    LAST_RESULTS = res
    acc = np.zeros((D, B * S), np.float32)
    for rmap in res.results:
        acc += rmap["outp"]
    return np.ascontiguousarray(acc.T).reshape(B, S, D)


if __name__ == "__main__":
    rng = np.random.default_rng(0)
    scale = 1.0 / np.sqrt(D)
    x = rng.standard_normal((B, S, D), dtype=np.float32)
    Wq = rng.standard_normal((H, E, D), dtype=np.float32) * scale
    Wk = rng.standard_normal((H, E, D), dtype=np.float32) * scale
    Wv = rng.standard_normal((H, E, D), dtype=np.float32) * scale
    Wo = rng.standard_normal((D, D), dtype=np.float32) * scale
    out = kernel(x, Wq, Wk, Wv, Wo)
    print(out.shape, out.dtype, float(np.abs(out).max()))
